# revision 27
# baseline (speedup 1.0000x reference)
"""GCN aggregator kernel for 8 Trainium2 NeuronCores (Bass/Tile).

Computes: out = D_r^{-1/2} M D_c^{-1/2} E[unique_ids]  where M is the
[B, U] 0/1 neighbor mask built from neigh_cols (duplicate (row, col)
pairs collapse to 1).

Sparse decomposition (exact):
  f[b,k]   = 1 if k is the first position in row b with value neigh_cols[b,k]
  row_cnt  = sum_k f[b,k]            (distinct neighbors per row)
  col_cnt  = scatter-add of f by u   (rows containing u; global over B)
  out[b]   = sum_k f[b,k] * rsqrt(row_cnt[b]) * icn[u] * E[unique_ids[u]],
             u = neigh_cols[b,k],  icn[u] = rsqrt(max(col_cnt[u], 1))

Sharding: B is split 8 ways (512 rows/core). The scaled gathered matrix
E'[u] = icn[u] * embed_table[unique_ids[u]] is built U-sharded (4096
rows/core) and AllGathered; col_cnt partials are AllReduced.

Per-core layouts:
  row b = 4p + t             (p = partition, t = row-tile 0..3)
  u     = 128*hi + lo        (histogram psum indexed [lo, hi])
  gather pair order i = 128*(4k + t) + p, wrapped idx at [i%16, i//16]
"""

import os
import numpy as np
from contextlib import ExitStack

import concourse.tile as tile
from concourse import bass, bacc, mybir
from concourse.bass_utils import run_bass_kernel_spmd
from concourse.masks import make_identity

dt = mybir.dt
Alu = mybir.AluOpType
Act = mybir.ActivationFunctionType

B, K, U, V, D = 4096, 32, 32768, 100000, 128
NC = 8
BC = B // NC          # 512 rows per core
TPC = BC // 128       # 4 row-tiles per core
USH = U // NC         # 4096 unique ids per core (U-shard)
JW = USH // 128       # 32 shard columns
NPAIR = BC * K        # 16384 gathered pairs per core
GCH = 4               # main-gather chunks
CHG = NPAIR // GCH // 128   # 32 groups of 128 pairs per chunk

LAST_RESULTS = None   # test harness reads profiling info from here
_PROGRAM = None


def _build_program():
    skips = set(os.environ.get("GCN_SKIP", "").split(","))
    noag = os.environ.get("GCN_MODE", "ag") == "noag"
    nc = bacc.Bacc("TRN2", target_bir_lowering=False, debug=False, num_devices=NC)

    t_x = nc.dram_tensor("x", [BC, K], dt.int32, kind="ExternalInput").ap()
    t_ids = nc.dram_tensor("ids", [2 * U // 256, 128] if noag else [JW, 128],
                           dt.int32, kind="ExternalInput").ap()
    t_emb = nc.dram_tensor("emb", [V, D], dt.float32, kind="ExternalInput").ap()
    t_iota = nc.dram_tensor("iotaf", [128, 256], dt.float32, kind="ExternalInput").ap()
    t_c127 = nc.dram_tensor("c127", [128, TPC * K], dt.int32,
                            kind="ExternalInput").ap()
    t_hrows = nc.dram_tensor("hrows", [32, 1], dt.int32, kind="ExternalInput").ap()
    t_idxw = nc.dram_tensor("idxw", [128, NPAIR // 16], dt.int16,
                            kind="ExternalInput").ap()
    t_id128 = nc.dram_tensor("id128", [128, 128], dt.float32,
                             kind="ExternalInput").ap()
    t_out = nc.dram_tensor("out", [BC, D], dt.float32, kind="ExternalOutput").ap()

    # standalone DRAM scratch (offset-0 APs for collectives / indirect reads)
    t_cnt_in = nc.dram_tensor("cnt_in", [256, 128], dt.float32).ap()
    t_cnt_out = nc.dram_tensor("cnt_out", [256, 128], dt.float32,
                               addr_space="Shared").ap()
    t_cnt_rs = nc.dram_tensor("cnt_rs", [32, 128], dt.float32).ap()
    t_esh = nc.dram_tensor("esh", [USH, D], dt.float32).ap()
    t_efull = nc.dram_tensor("efull", [U, D], dt.float32, addr_space="Shared").ap()

    with tile.TileContext(nc) as tc, ExitStack() as ctx:
        sb = ctx.enter_context(tc.tile_pool(name="sb", bufs=1))
        sbd = ctx.enter_context(tc.tile_pool(name="sbd", bufs=6))
        gpool = ctx.enter_context(tc.tile_pool(name="gp", bufs=3))
        ps = ctx.enter_context(tc.tile_pool(name="ps", bufs=1, space="PSUM"))
        psd = ctx.enter_context(tc.tile_pool(name="psd", bufs=2, space="PSUM"))

        # ---------- loads ----------
        s_x = sb.tile([128, TPC, K], dt.int32)        # [p, t, k]; row b = 4p + t
        nc.sync.dma_start(s_x[:], t_x.rearrange("(p t) k -> p t k", t=TPC))
        s_iota = sb.tile([128, 256], dt.float32)
        nc.sync.dma_start(s_iota[:], t_iota)
        s_c127 = sb.tile([128, TPC, K], dt.int32)
        nc.sync.dma_start(s_c127[:], t_c127.rearrange("p (t k) -> p t k", t=TPC))
        if noag:
            s_idsr2 = sb.tile([128, 2, 128], dt.int32)
            nc.sync.dma_start(
                s_idsr2[:], t_ids.rearrange("(h p) c -> p h c", h=2))
        else:
            s_idsr = sb.tile([JW, 128], dt.int32)
            nc.sync.dma_start(s_idsr[:], t_ids)
        s_hrows = sb.tile([32, 1], dt.int32)
        nc.sync.dma_start(s_hrows[:], t_hrows)

        s_id128 = sb.tile([128, 128], dt.float32)
        nc.sync.dma_start(s_id128[:], t_id128)
        s_id32 = sb.tile([32, 32], dt.float32)
        nc.vector.tensor_copy(s_id32[:], s_id128[0:32, 0:32])

        # ---------- first-occurrence mask + row norm ----------
        s_xf = sb.tile([128, TPC, K], dt.float32)
        nc.vector.tensor_copy(s_xf[:], s_x[:])

        s_dup = sb.tile([128, TPC, K], dt.float32)
        nc.vector.memset(s_dup[:], 0.0)
        s_eq = sb.tile([128, TPC, K], dt.float32)
        for sh in range(1, (2 if "focc" in skips else K)):
            w = K - sh
            nc.vector.tensor_tensor(
                out=s_eq[:, :, :w], in0=s_xf[:, :, sh:K], in1=s_xf[:, :, 0:w],
                op=Alu.is_equal)
            nc.vector.tensor_tensor(
                out=s_dup[:, :, sh:K], in0=s_dup[:, :, sh:K], in1=s_eq[:, :, :w],
                op=Alu.add)
        s_f = sb.tile([128, TPC, K], dt.float32)
        nc.vector.tensor_scalar(
            out=s_f[:], in0=s_dup[:], scalar1=0.0, scalar2=None, op0=Alu.is_equal)

        s_rcnt = sb.tile([128, TPC], dt.float32)
        nc.vector.tensor_reduce(
            out=s_rcnt[:], in_=s_f[:], axis=mybir.AxisListType.X, op=Alu.add)
        s_rsq = sb.tile([128, TPC], dt.float32)
        nc.scalar.activation(out=s_rsq[:], in_=s_rcnt[:], func=Act.Sqrt)
        s_rn = sb.tile([128, TPC], dt.float32)
        nc.vector.reciprocal(out=s_rn[:], in_=s_rsq[:])
        # s-weights: f * rn  (f32, exact)
        s_s = sb.tile([128, TPC, K], dt.float32)
        nc.vector.tensor_tensor(
            out=s_s[:], in0=s_f[:],
            in1=s_rn[:].to_broadcast([128, TPC, K]),
            op=Alu.mult)

        # ---------- lo/hi split of u (u = hi*128 + lo) ----------
        s_lo = sb.tile([128, TPC, K], dt.int32)
        nc.vector.tensor_tensor(
            out=s_lo[:], in0=s_x[:], in1=s_c127[:], op=Alu.bitwise_and)
        s_lof = sb.tile([128, TPC, K], dt.float32)
        nc.vector.tensor_copy(s_lof[:], s_lo[:])
        s_hif = sb.tile([128, TPC, K], dt.float32)
        nc.vector.tensor_tensor(
            out=s_hif[:], in0=s_xf[:], in1=s_lof[:], op=Alu.subtract)
        s_hifs = sb.tile([128, TPC, K], dt.float32)
        nc.vector.tensor_scalar(
            out=s_hifs[:], in0=s_hif[:], scalar1=1.0 / 128.0, scalar2=None,
            op0=Alu.mult)

        # ---------- col-count histogram: psum[lo, hi] += LO^T @ (HI*f) ----------
        p_hist = ps.tile([128, 256], dt.float32, space="PSUM")
        nchunk = TPC * K
        if "hist" in skips:
            nchunk = 1
        for ci in range(nchunk):
            t, k = divmod(ci, K)
            lo16 = sbd.tile([128, 128], dt.bfloat16, tag="lo16")
            nc.vector.tensor_scalar(
                out=lo16[:], in0=s_iota[:, 0:128], scalar1=s_lof[:, t, k:k + 1],
                scalar2=None, op0=Alu.is_equal)
            hi16 = sbd.tile([128, 256], dt.bfloat16, tag="hi16")
            nc.vector.tensor_scalar(
                out=hi16[:], in0=s_iota[:, 0:256], scalar1=s_hifs[:, t, k:k + 1],
                scalar2=s_f[:, t, k:k + 1], op0=Alu.is_equal, op1=Alu.mult)
            nc.tensor.matmul(
                p_hist[:], lhsT=lo16[:], rhs=hi16[:],
                start=(ci == 0), stop=(ci == nchunk - 1))

        s_hist = sb.tile([128, 256], dt.float32)
        nc.vector.tensor_copy(s_hist[:], p_hist[:])

        # transpose to [hi, lo] u-major layout for the AllReduce bounce
        if not noag:
            s_tr2 = sb.tile([128, 2, 128], dt.float32)
            for h in range(2):
                p_tr = psd.tile([128, 128], dt.float32, space="PSUM", tag="ptr")
                nc.tensor.transpose(
                    out=p_tr[:], in_=s_hist[:, 128 * h:128 * (h + 1)],
                    identity=s_id128[:])
                nc.vector.tensor_copy(s_tr2[:, h, :], p_tr[:])
            nc.sync.dma_start(
                t_cnt_in.rearrange("(h p) c -> p h c", h=2), s_tr2[:])

        if noag:
            nc.sync.dma_start(t_cnt_in[0:128, :].rearrange("p c -> p c"), s_hist[:, 0:128])
            nc.sync.dma_start(t_cnt_in[128:256, :], s_hist[:, 128:256])
        if "coll" in skips:
            nc.gpsimd.dma_start(t_cnt_out[:], t_cnt_in[:])
        elif noag:
            nc.gpsimd.collective_compute(
                "AllReduce", Alu.add, replica_groups=[list(range(NC))],
                ins=[t_cnt_in], outs=[t_cnt_out])
        else:
            nc.gpsimd.collective_compute(
                "ReduceScatter", Alu.add, replica_groups=[list(range(NC))],
                ins=[t_cnt_in], outs=[t_cnt_rs])

        if noag:
            s_cfull = sb.tile([128, 256], dt.float32)
            nc.sync.dma_start(s_cfull[:, 0:128], t_cnt_out[0:128, :])
            nc.sync.dma_start(s_cfull[:, 128:256], t_cnt_out[128:256, :])
            s_icsq = sb.tile([128, 256], dt.float32)
            nc.vector.tensor_scalar(
                out=s_icsq[:], in0=s_cfull[:], scalar1=1.0, scalar2=None,
                op0=Alu.max)
            nc.scalar.activation(out=s_icsq[:], in_=s_icsq[:], func=Act.Sqrt)
            s_icn = sb.tile([128, 256], dt.float32)
            nc.vector.reciprocal(out=s_icn[:], in_=s_icsq[:])
        else:
            s_csh = sb.tile([32, 128], dt.float32)
            nc.sync.dma_start(s_csh[:], t_cnt_rs)
            p_cs = ps.tile([128, 32], dt.float32, space="PSUM")
            nc.tensor.transpose(out=p_cs[:], in_=s_csh[:], identity=s_id32[:])
            s_icsq = sb.tile([128, JW], dt.float32)
            nc.vector.tensor_scalar(
                out=s_icsq[:], in0=p_cs[:], scalar1=1.0, scalar2=None, op0=Alu.max)
            nc.scalar.activation(out=s_icsq[:], in_=s_icsq[:], func=Act.Sqrt)
            s_icn = sb.tile([128, JW], dt.float32)
            nc.vector.reciprocal(out=s_icn[:], in_=s_icsq[:])

        # ---------- build E' (shard or full): E'[u] = icn[u] * emb[ids[u]] ----------
        if noag:
            # transpose full ids [2x 128,128] -> s_idt [128, 256]
            s_idsf2 = sb.tile([128, 2, 128], dt.float32)
            nc.vector.tensor_copy(s_idsf2[:], s_idsr2[:])
            s_idt2 = sb.tile([128, 256], dt.int32)
            for h in range(2):
                p_idt = psd.tile([128, 128], dt.float32, space="PSUM", tag="pidt")
                nc.tensor.transpose(
                    out=p_idt[:], in_=s_idsf2[:, h, :], identity=s_id128[:])
                nc.vector.tensor_copy(s_idt2[:, 128 * h:128 * (h + 1)], p_idt[:])
            nebuild = 1 if "ebuild" in skips else 256
            for w in range((nebuild + 31) // 32):
                s_eb = gpool.tile([128, JW, D], dt.float32, tag="ebw")
                for jj in range(min(32, nebuild - 32 * w)):
                    col = 32 * w + jj
                    nc.gpsimd.indirect_dma_start(
                        out=s_eb[:, jj, :], out_offset=None, in_=t_emb,
                        in_offset=bass.IndirectOffsetOnAxis(
                            ap=s_idt2[:, col:col + 1], axis=0))
                for jj in range(min(32, nebuild - 32 * w)):
                    col = 32 * w + jj
                    nc.vector.tensor_tensor(
                        out=s_eb[:, jj, :], in0=s_eb[:, jj, :],
                        in1=s_icn[:, col:col + 1].to_broadcast([128, D]),
                        op=Alu.mult)
                    nc.sync.dma_start(
                        t_efull[128 * col:128 * (col + 1), :], s_eb[:, jj, :])
        else:
            s_idsf = sb.tile([JW, 128], dt.float32)
            nc.vector.tensor_copy(s_idsf[:], s_idsr[:])
            p_idt = ps.tile([128, JW], dt.float32, space="PSUM")
            nc.tensor.transpose(out=p_idt[:], in_=s_idsf[:], identity=s_id32[:])
            s_idt = sb.tile([128, JW], dt.int32)
            nc.vector.tensor_copy(s_idt[:], p_idt[:])

            s_eb = sb.tile([128, JW, D], dt.float32)
            for j in range(1 if "ebuild" in skips else JW):
                nc.gpsimd.indirect_dma_start(
                    out=s_eb[:, j, :], out_offset=None, in_=t_emb,
                    in_offset=bass.IndirectOffsetOnAxis(ap=s_idt[:, j:j + 1], axis=0))
            nc.vector.tensor_tensor(
                out=s_eb[:], in0=s_eb[:],
                in1=s_icn[:].to_broadcast([128, JW, D]),
                op=Alu.mult)
            nc.sync.dma_start(
                t_esh.rearrange("(j p) d -> p j d", p=128), s_eb[:])

            if "coll" in skips:
                nc.gpsimd.dma_start(t_efull[0:USH, :], t_esh[:])
            else:
                nc.gpsimd.collective_compute(
                    "AllGather", Alu.bypass, replica_groups=[list(range(NC))],
                    ins=[t_esh], outs=[t_efull])

        # ---------- wrapped int16 index array (host-prepermuted layout) ----------
        s_idxw = sb.tile([128, NPAIR // 16], dt.int16)
        nc.sync.dma_start(s_idxw[:], t_idxw)

        # ---------- main gather + weighted sum ----------
        s_acc = sb.tile([128, TPC, D], dt.float32)
        if "wsum" in skips or "gather" in skips:
            nc.vector.memset(s_acc[:], 0.0)
        kpch = K // GCH
        for ch in range(0 if "gather" in skips else GCH):
            s_g = gpool.tile([128, CHG, D], dt.float32, tag="gch")
            nc.gpsimd.dma_gather(
                out_ap=s_g[:], in_ap=t_efull,
                idxs_ap=s_idxw[:, (NPAIR // 16 // GCH) * ch:
                               (NPAIR // 16 // GCH) * (ch + 1)],
                num_idxs=NPAIR // GCH, num_idxs_reg=NPAIR // GCH, elem_size=D,
                single_packet=False)
            for dk in range(0 if "wsum" in skips else kpch):
                k = kpch * ch + dk
                for t in range(TPC):
                    g = 4 * dk + t
                    if ch == 0 and dk == 0:
                        nc.vector.tensor_scalar(
                            out=s_acc[:, t, :], in0=s_g[:, g, :],
                            scalar1=s_s[:, t, k:k + 1], scalar2=None,
                            op0=Alu.mult)
                    else:
                        nc.vector.scalar_tensor_tensor(
                            out=s_acc[:, t, :], in0=s_g[:, g, :],
                            scalar=s_s[:, t, k:k + 1], in1=s_acc[:, t, :],
                            op0=Alu.mult, op1=Alu.add)

        nc.sync.dma_start(t_out.rearrange("(p t) d -> p t d", t=TPC), s_acc[:])

    nc.compile()
    return nc


def _get_program():
    global _PROGRAM
    if _PROGRAM is None:
        _PROGRAM = _build_program()
    return _PROGRAM


def _make_in_maps(neigh_cols, unique_ids, embed_table):
    neigh_cols = np.ascontiguousarray(np.asarray(neigh_cols, dtype=np.int32))
    unique_ids = np.ascontiguousarray(np.asarray(unique_ids, dtype=np.int32))
    embed_table = np.ascontiguousarray(np.asarray(embed_table, dtype=np.float32))
    iotaf = np.broadcast_to(
        np.arange(256, dtype=np.float32), (128, 256)).copy()
    c127 = np.full((128, TPC * K), 127, np.int32)

    # wrapped gather indices: pair order i = 128*(4k + t) + p, b = 4p + t
    i = np.arange(NPAIR)
    p = i % 128
    g = i // 128
    t = g % TPC
    k = g // TPC

    in_maps = []
    for c in range(NC):
        noag = os.environ.get("GCN_MODE", "ag") == "noag"
        xs = neigh_cols[BC * c:BC * (c + 1)]
        vals = xs[TPC * p + t, k].astype(np.int16)
        idxw = np.zeros((16, NPAIR // 16), np.int16)
        idxw[i % 16, i // 16] = vals
        idxw = np.tile(idxw, (8, 1))
        in_maps.append({
            "x": xs,
            "idxw": idxw,
            "id128": np.eye(128, dtype=np.float32),
            "ids": (unique_ids.reshape(2 * U // 256, 128) if noag else
                    unique_ids[USH * c:USH * (c + 1)].reshape(JW, 128)),
            "emb": embed_table,
            "iotaf": iotaf,
            "c127": c127,
            "hrows": np.arange(32 * c, 32 * (c + 1), dtype=np.int32).reshape(32, 1),
        })

    return in_maps


def kernel(neigh_cols, unique_ids, embed_table):
    global LAST_RESULTS
    nc = _get_program()
    in_maps = _make_in_maps(neigh_cols, unique_ids, embed_table)
    trace = bool(int(os.environ.get("GCN_TRACE", "0")))
    res = run_bass_kernel_spmd(nc, in_maps, list(range(NC)), trace=trace)
    LAST_RESULTS = res
    out = np.concatenate([res.results[c]["out"] for c in range(NC)], axis=0)
    return out.astype(np.float32)


def bench_exec(inputs, iters=12):
    """Steady-state wall times (us) of the compiled NEFF via a reusable
    sharded jit with device-resident inputs. Excludes compile; includes
    per-call dispatch overhead of the runtime."""
    import time
    import jax
    from jax.sharding import Mesh, PartitionSpec, NamedSharding
    from jax.experimental.shard_map import shard_map
    from concourse.bass2jax import (_bass_exec_p, partition_id_tensor,
                                    install_neuronx_cc_hook)

    nc = _get_program()
    install_neuronx_cc_hook()
    in_maps = _make_in_maps(**inputs)

    partition_name = (nc.partition_id_tensor.name
                      if nc.partition_id_tensor else None)
    in_names, out_names, out_avals, zero_outs = [], [], [], []
    for alloc in nc.m.functions[0].allocations:
        if not isinstance(alloc, mybir.MemoryLocationSet):
            continue
        name = alloc.memorylocations[0].name
        if alloc.kind == "ExternalInput":
            if name != partition_name:
                in_names.append(name)
        elif alloc.kind == "ExternalOutput":
            out_names.append(name)
            shape = tuple(alloc.tensor_shape)
            npdt = dt.np(alloc.dtype)
            out_avals.append(jax.core.ShapedArray(shape, npdt))
            zero_outs.append(np.zeros(shape, npdt))
    n_params = len(in_names)
    all_names = in_names + out_names + ([partition_name] if partition_name else [])

    def _body(*args):
        operands = list(args)
        if partition_name is not None:
            operands.append(partition_id_tensor())
        return tuple(_bass_exec_p.bind(
            *operands, out_avals=tuple(out_avals), in_names=tuple(all_names),
            out_names=tuple(out_names), lowering_input_output_aliases=(),
            sim_require_finite=True, sim_require_nnan=True, nc=nc))

    devices = jax.devices()[:NC]
    mesh = Mesh(np.asarray(devices), ("core",))
    sharded = jax.jit(
        shard_map(_body, mesh=mesh,
                  in_specs=(PartitionSpec("core"),) * (n_params + len(out_names)),
                  out_specs=(PartitionSpec("core"),) * len(out_names),
                  check_rep=False),
        keep_unused=True)
    sh = NamedSharding(mesh, PartitionSpec("core"))
    concat_in = [jax.device_put(
        np.concatenate([np.asarray(in_maps[c][nm]) for c in range(NC)], axis=0),
        sh) for nm in in_names]
    concat_zero = [jax.device_put(
        np.zeros((NC * z.shape[0], *z.shape[1:]), z.dtype), sh)
        for z in zero_outs]
    out = sharded(*concat_in, *concat_zero)
    jax.block_until_ready(out)
    times = []
    for _ in range(iters):
        t0 = time.perf_counter()
        out = sharded(*concat_in, *concat_zero)
        jax.block_until_ready(out)
        times.append((time.perf_counter() - t0) * 1e6)
    return sorted(times)


def modeled_time_ns():
    """Single-core device-occupancy model of the program (cost-model sim)."""
    from concourse.timeline_sim import TimelineSim
    return TimelineSim(_get_program(), trace=False).simulate()


# revision 31
# speedup vs baseline: 1.2766x; 1.2766x over previous
"""GCN aggregator kernel for 8 Trainium2 NeuronCores (Bass/Tile).

Computes: out = D_r^{-1/2} M D_c^{-1/2} E[unique_ids]  where M is the
[B, U] 0/1 neighbor mask built from neigh_cols (duplicate (row, col)
pairs collapse to 1).

Sparse decomposition (exact):
  f[b,k]   = 1 if k is the first position in row b with value neigh_cols[b,k]
  row_cnt  = sum_k f[b,k]            (distinct neighbors per row)
  col_cnt  = scatter-add of f by u   (rows containing u; global over B)
  out[b]   = sum_k f[b,k] * rsqrt(row_cnt[b]) * icn[u] * E[unique_ids[u]],
             u = neigh_cols[b,k],  icn[u] = rsqrt(max(col_cnt[u], 1))

Sharding: B is split 8 ways (512 rows/core). The scaled gathered matrix
E'[u] = icn[u] * embed_table[unique_ids[u]] is built U-sharded (4096
rows/core) and AllGathered; col_cnt partials are AllReduced.

Per-core layouts:
  row b = 4p + t             (p = partition, t = row-tile 0..3)
  u     = 128*hi + lo        (histogram psum indexed [lo, hi])
  gather pair order i = 128*(4k + t) + p, wrapped idx at [i%16, i//16]
"""

import os
import numpy as np
from contextlib import ExitStack

import concourse.tile as tile
from concourse import bass, bacc, mybir
from concourse.bass_utils import run_bass_kernel_spmd
from concourse.masks import make_identity

dt = mybir.dt
Alu = mybir.AluOpType
Act = mybir.ActivationFunctionType

B, K, U, V, D = 4096, 32, 32768, 100000, 128
NC = 8
BC = B // NC          # 512 rows per core
TPC = BC // 128       # 4 row-tiles per core
USH = U // NC         # 4096 unique ids per core (U-shard)
JW = USH // 128       # 32 shard columns
NPAIR = BC * K        # 16384 gathered pairs per core
GCH = 4               # main-gather chunks
CHG = NPAIR // GCH // 128   # 32 groups of 128 pairs per chunk

LAST_RESULTS = None   # test harness reads profiling info from here
_PROGRAM = None


def _build_program():
    skips = set(os.environ.get("GCN_SKIP", "").split(","))
    noag = os.environ.get("GCN_MODE", "ag") == "noag"
    nc = bacc.Bacc("TRN2", target_bir_lowering=False, debug=False, num_devices=NC)

    t_x = nc.dram_tensor("x", [BC, K], dt.int32, kind="ExternalInput").ap()
    t_ids = nc.dram_tensor("ids", [2 * U // 256, 128] if noag else [JW, 128],
                           dt.int32, kind="ExternalInput").ap()
    t_emb = nc.dram_tensor("emb", [V, D], dt.float32, kind="ExternalInput").ap()
    t_iota = nc.dram_tensor("iotaf", [128, 256], dt.float32, kind="ExternalInput").ap()
    t_c127 = nc.dram_tensor("c127", [128, TPC * K], dt.int32,
                            kind="ExternalInput").ap()
    t_hrows = nc.dram_tensor("hrows", [32, 1], dt.int32, kind="ExternalInput").ap()
    t_idxw = nc.dram_tensor("idxw", [128, NPAIR // 16], dt.int16,
                            kind="ExternalInput").ap()
    t_id128 = nc.dram_tensor("id128", [128, 128], dt.float32,
                             kind="ExternalInput").ap()
    t_out = nc.dram_tensor("out", [BC, D], dt.float32, kind="ExternalOutput").ap()

    # standalone DRAM scratch (offset-0 APs for collectives / indirect reads)
    t_cnt_in = nc.dram_tensor("cnt_in", [256, 128], dt.float32).ap()
    t_cnt_out = nc.dram_tensor("cnt_out", [256, 128], dt.float32,
                               addr_space="Shared").ap()
    t_cnt_rs = nc.dram_tensor("cnt_rs", [32, 128], dt.float32).ap()
    t_esh = nc.dram_tensor("esh", [USH, D], dt.float32).ap()
    t_efull = nc.dram_tensor("efull", [U, D], dt.float32, addr_space="Shared").ap()

    with tile.TileContext(nc) as tc, ExitStack() as ctx:
        sb = ctx.enter_context(tc.tile_pool(name="sb", bufs=1))
        sbd = ctx.enter_context(tc.tile_pool(name="sbd", bufs=6))
        gpool = ctx.enter_context(tc.tile_pool(name="gp", bufs=3))
        ps = ctx.enter_context(tc.tile_pool(name="ps", bufs=1, space="PSUM"))
        psd = ctx.enter_context(tc.tile_pool(name="psd", bufs=2, space="PSUM"))

        # ---------- loads ----------
        s_x = sb.tile([128, TPC, K], dt.int32)        # [p, t, k]; row b = 4p + t
        nc.sync.dma_start(s_x[:], t_x.rearrange("(p t) k -> p t k", t=TPC))
        s_iota = sb.tile([128, 256], dt.float32)
        nc.sync.dma_start(s_iota[:], t_iota)
        s_c127 = sb.tile([128, TPC, K], dt.int32)
        nc.sync.dma_start(s_c127[:], t_c127.rearrange("p (t k) -> p t k", t=TPC))
        if noag:
            s_idsr2 = sb.tile([128, 2, 128], dt.int32)
            nc.sync.dma_start(
                s_idsr2[:], t_ids.rearrange("(h p) c -> p h c", h=2))
        else:
            s_idsr = sb.tile([JW, 128], dt.int32)
            nc.sync.dma_start(s_idsr[:], t_ids)
        s_hrows = sb.tile([32, 1], dt.int32)
        nc.sync.dma_start(s_hrows[:], t_hrows)

        s_id128 = sb.tile([128, 128], dt.float32)
        nc.sync.dma_start(s_id128[:], t_id128)
        s_id32 = sb.tile([32, 32], dt.float32)
        nc.vector.tensor_copy(s_id32[:], s_id128[0:32, 0:32])

        # ---------- first-occurrence mask + row norm ----------
        s_xf = sb.tile([128, TPC, K], dt.float32)
        nc.vector.tensor_copy(s_xf[:], s_x[:])

        s_dup = sb.tile([128, TPC, K], dt.float32)
        nc.vector.memset(s_dup[:], 0.0)
        s_eq = sb.tile([128, TPC, K], dt.float32)
        for sh in range(1, (2 if "focc" in skips else K)):
            w = K - sh
            nc.vector.tensor_tensor(
                out=s_eq[:, :, :w], in0=s_xf[:, :, sh:K], in1=s_xf[:, :, 0:w],
                op=Alu.is_equal)
            nc.vector.tensor_tensor(
                out=s_dup[:, :, sh:K], in0=s_dup[:, :, sh:K], in1=s_eq[:, :, :w],
                op=Alu.add)
        s_f = sb.tile([128, TPC, K], dt.float32)
        nc.vector.tensor_scalar(
            out=s_f[:], in0=s_dup[:], scalar1=0.0, scalar2=None, op0=Alu.is_equal)

        s_rcnt = sb.tile([128, TPC], dt.float32)
        nc.vector.tensor_reduce(
            out=s_rcnt[:], in_=s_f[:], axis=mybir.AxisListType.X, op=Alu.add)
        s_rsq = sb.tile([128, TPC], dt.float32)
        nc.scalar.activation(out=s_rsq[:], in_=s_rcnt[:], func=Act.Sqrt)
        s_rn = sb.tile([128, TPC], dt.float32)
        nc.vector.reciprocal(out=s_rn[:], in_=s_rsq[:])
        # s-weights: f * rn  (f32, exact)
        s_s = sb.tile([128, TPC, K], dt.float32)
        nc.vector.tensor_tensor(
            out=s_s[:], in0=s_f[:],
            in1=s_rn[:].to_broadcast([128, TPC, K]),
            op=Alu.mult)

        # ---------- lo/hi split of u (u = hi*128 + lo) ----------
        s_lo = sb.tile([128, TPC, K], dt.int32)
        nc.vector.tensor_tensor(
            out=s_lo[:], in0=s_x[:], in1=s_c127[:], op=Alu.bitwise_and)
        s_lof = sb.tile([128, TPC, K], dt.float32)
        nc.vector.tensor_copy(s_lof[:], s_lo[:])
        s_hif = sb.tile([128, TPC, K], dt.float32)
        nc.vector.tensor_tensor(
            out=s_hif[:], in0=s_xf[:], in1=s_lof[:], op=Alu.subtract)
        s_hifs = sb.tile([128, TPC, K], dt.float32)
        nc.vector.tensor_scalar(
            out=s_hifs[:], in0=s_hif[:], scalar1=1.0 / 128.0, scalar2=None,
            op0=Alu.mult)

        # ---------- col-count histogram: psum[lo, hi] += LO^T @ (HI*f) ----------
        p_hist = ps.tile([128, 256], dt.float32, space="PSUM")
        nchunk = TPC * K
        if "hist" in skips:
            nchunk = 1
        for ci in range(nchunk):
            t, k = divmod(ci, K)
            lo16 = sbd.tile([128, 128], dt.bfloat16, tag="lo16")
            nc.vector.tensor_scalar(
                out=lo16[:], in0=s_iota[:, 0:128], scalar1=s_lof[:, t, k:k + 1],
                scalar2=None, op0=Alu.is_equal)
            hi16 = sbd.tile([128, 256], dt.bfloat16, tag="hi16")
            nc.vector.tensor_scalar(
                out=hi16[:], in0=s_iota[:, 0:256], scalar1=s_hifs[:, t, k:k + 1],
                scalar2=s_f[:, t, k:k + 1], op0=Alu.is_equal, op1=Alu.mult)
            nc.tensor.matmul(
                p_hist[:], lhsT=lo16[:], rhs=hi16[:],
                start=(ci == 0), stop=(ci == nchunk - 1))

        s_hist = sb.tile([128, 256], dt.float32)
        nc.vector.tensor_copy(s_hist[:], p_hist[:])

        # transpose to [hi, lo] u-major layout for the AllReduce bounce
        if not noag:
            s_tr2 = sb.tile([128, 2, 128], dt.float32)
            for h in range(2):
                p_tr = psd.tile([128, 128], dt.float32, space="PSUM", tag="ptr")
                nc.tensor.transpose(
                    out=p_tr[:], in_=s_hist[:, 128 * h:128 * (h + 1)],
                    identity=s_id128[:])
                nc.vector.tensor_copy(s_tr2[:, h, :], p_tr[:])
            nc.sync.dma_start(
                t_cnt_in.rearrange("(h p) c -> p h c", h=2), s_tr2[:])

        if noag:
            nc.sync.dma_start(t_cnt_in[0:128, :].rearrange("p c -> p c"), s_hist[:, 0:128])
            nc.sync.dma_start(t_cnt_in[128:256, :], s_hist[:, 128:256])
        if "coll" in skips:
            nc.gpsimd.dma_start(t_cnt_out[:], t_cnt_in[:])
        elif noag:
            nc.gpsimd.collective_compute(
                "AllReduce", Alu.add, replica_groups=[list(range(NC))],
                ins=[t_cnt_in], outs=[t_cnt_out])
        else:
            nc.gpsimd.collective_compute(
                "ReduceScatter", Alu.add, replica_groups=[list(range(NC))],
                ins=[t_cnt_in], outs=[t_cnt_rs])

        if noag:
            s_cfull = sb.tile([128, 256], dt.float32)
            nc.sync.dma_start(s_cfull[:, 0:128], t_cnt_out[0:128, :])
            nc.sync.dma_start(s_cfull[:, 128:256], t_cnt_out[128:256, :])
            s_icsq = sb.tile([128, 256], dt.float32)
            nc.vector.tensor_scalar(
                out=s_icsq[:], in0=s_cfull[:], scalar1=1.0, scalar2=None,
                op0=Alu.max)
            nc.scalar.activation(out=s_icsq[:], in_=s_icsq[:], func=Act.Sqrt)
            s_icn = sb.tile([128, 256], dt.float32)
            nc.vector.reciprocal(out=s_icn[:], in_=s_icsq[:])
        else:
            s_csh = sb.tile([32, 128], dt.float32)
            nc.sync.dma_start(s_csh[:], t_cnt_rs)
            p_cs = ps.tile([128, 32], dt.float32, space="PSUM")
            nc.tensor.transpose(out=p_cs[:], in_=s_csh[:], identity=s_id32[:])
            s_icsq = sb.tile([128, JW], dt.float32)
            nc.vector.tensor_scalar(
                out=s_icsq[:], in0=p_cs[:], scalar1=1.0, scalar2=None, op0=Alu.max)
            nc.scalar.activation(out=s_icsq[:], in_=s_icsq[:], func=Act.Sqrt)
            s_icn = sb.tile([128, JW], dt.float32)
            nc.vector.reciprocal(out=s_icn[:], in_=s_icsq[:])

        # ---------- build E' (shard or full): E'[u] = icn[u] * emb[ids[u]] ----------
        if noag:
            # transpose full ids [2x 128,128] -> s_idt [128, 256]
            s_idsf2 = sb.tile([128, 2, 128], dt.float32)
            nc.vector.tensor_copy(s_idsf2[:], s_idsr2[:])
            s_idt2 = sb.tile([128, 256], dt.int32)
            for h in range(2):
                p_idt = psd.tile([128, 128], dt.float32, space="PSUM", tag="pidt")
                nc.tensor.transpose(
                    out=p_idt[:], in_=s_idsf2[:, h, :], identity=s_id128[:])
                nc.vector.tensor_copy(s_idt2[:, 128 * h:128 * (h + 1)], p_idt[:])
            nebuild = 1 if "ebuild" in skips else 256
            for w in range((nebuild + 31) // 32):
                s_eb = gpool.tile([128, JW, D], dt.float32, tag="ebw")
                for jj in range(min(32, nebuild - 32 * w)):
                    col = 32 * w + jj
                    nc.gpsimd.indirect_dma_start(
                        out=s_eb[:, jj, :], out_offset=None, in_=t_emb,
                        in_offset=bass.IndirectOffsetOnAxis(
                            ap=s_idt2[:, col:col + 1], axis=0))
                for jj in range(min(32, nebuild - 32 * w)):
                    col = 32 * w + jj
                    nc.vector.tensor_tensor(
                        out=s_eb[:, jj, :], in0=s_eb[:, jj, :],
                        in1=s_icn[:, col:col + 1].to_broadcast([128, D]),
                        op=Alu.mult)
                    nc.sync.dma_start(
                        t_efull[128 * col:128 * (col + 1), :], s_eb[:, jj, :])
        else:
            s_idsf = sb.tile([JW, 128], dt.float32)
            nc.vector.tensor_copy(s_idsf[:], s_idsr[:])
            p_idt = ps.tile([128, JW], dt.float32, space="PSUM")
            nc.tensor.transpose(out=p_idt[:], in_=s_idsf[:], identity=s_id32[:])
            s_idt = sb.tile([128, JW], dt.int32)
            nc.vector.tensor_copy(s_idt[:], p_idt[:])

            s_eb = sb.tile([128, JW, D], dt.float32)
            for j in range(1 if "ebuild" in skips else JW):
                nc.gpsimd.indirect_dma_start(
                    out=s_eb[:, j, :], out_offset=None, in_=t_emb,
                    in_offset=bass.IndirectOffsetOnAxis(ap=s_idt[:, j:j + 1], axis=0))
            nc.vector.tensor_tensor(
                out=s_eb[:], in0=s_eb[:],
                in1=s_icn[:].to_broadcast([128, JW, D]),
                op=Alu.mult)
            nc.sync.dma_start(
                t_esh.rearrange("(j p) d -> p j d", p=128), s_eb[:])

            if "coll" in skips:
                nc.gpsimd.dma_start(t_efull[0:USH, :], t_esh[:])
            else:
                nc.gpsimd.collective_compute(
                    "AllGather", Alu.bypass, replica_groups=[list(range(NC))],
                    ins=[t_esh], outs=[t_efull])

        # ---------- wrapped int16 index array (host-prepermuted layout) ----------
        s_idxw = sb.tile([128, NPAIR // 16], dt.int16)
        nc.sync.dma_start(s_idxw[:], t_idxw)

        # ---------- main gather + weighted sum ----------
        s_acc = sb.tile([128, TPC, D], dt.float32)
        if "wsum" in skips or "gather" in skips:
            nc.vector.memset(s_acc[:], 0.0)
        kpch = K // GCH
        for ch in range(0 if "gather" in skips else GCH):
            s_g = gpool.tile([128, CHG, D], dt.float32, tag="gch")
            nc.gpsimd.dma_gather(
                out_ap=s_g[:], in_ap=t_efull,
                idxs_ap=s_idxw[:, (NPAIR // 16 // GCH) * ch:
                               (NPAIR // 16 // GCH) * (ch + 1)],
                num_idxs=NPAIR // GCH, num_idxs_reg=NPAIR // GCH, elem_size=D,
                single_packet=False)
            for dk in range(0 if "wsum" in skips else kpch):
                k = kpch * ch + dk
                for t in range(TPC):
                    g = 4 * dk + t
                    if ch == 0 and dk == 0:
                        nc.vector.tensor_scalar(
                            out=s_acc[:, t, :], in0=s_g[:, g, :],
                            scalar1=s_s[:, t, k:k + 1], scalar2=None,
                            op0=Alu.mult)
                    else:
                        nc.vector.scalar_tensor_tensor(
                            out=s_acc[:, t, :], in0=s_g[:, g, :],
                            scalar=s_s[:, t, k:k + 1], in1=s_acc[:, t, :],
                            op0=Alu.mult, op1=Alu.add)

        nc.sync.dma_start(t_out.rearrange("(p t) d -> p t d", t=TPC), s_acc[:])

    nc.compile()
    return nc


def _get_program():
    global _PROGRAM
    if _PROGRAM is None:
        _PROGRAM = _build_program()
    return _PROGRAM


def _make_in_maps(neigh_cols, unique_ids, embed_table):
    neigh_cols = np.ascontiguousarray(np.asarray(neigh_cols, dtype=np.int32))
    unique_ids = np.ascontiguousarray(np.asarray(unique_ids, dtype=np.int32))
    embed_table = np.ascontiguousarray(np.asarray(embed_table, dtype=np.float32))
    iotaf = np.broadcast_to(
        np.arange(256, dtype=np.float32), (128, 256)).copy()
    c127 = np.full((128, TPC * K), 127, np.int32)

    # wrapped gather indices: pair order i = 128*(4k + t) + p, b = 4p + t
    i = np.arange(NPAIR)
    p = i % 128
    g = i // 128
    t = g % TPC
    k = g // TPC

    in_maps = []
    for c in range(NC):
        noag = os.environ.get("GCN_MODE", "ag") == "noag"
        xs = neigh_cols[BC * c:BC * (c + 1)]
        vals = xs[TPC * p + t, k].astype(np.int16)
        idxw = np.zeros((16, NPAIR // 16), np.int16)
        idxw[i % 16, i // 16] = vals
        idxw = np.tile(idxw, (8, 1))
        in_maps.append({
            "x": xs,
            "idxw": idxw,
            "id128": np.eye(128, dtype=np.float32),
            "ids": (unique_ids.reshape(2 * U // 256, 128) if noag else
                    unique_ids[USH * c:USH * (c + 1)].reshape(JW, 128)),
            "emb": embed_table,
            "iotaf": iotaf,
            "c127": c127,
            "hrows": np.arange(32 * c, 32 * (c + 1), dtype=np.int32).reshape(32, 1),
        })

    return in_maps


def kernel(neigh_cols, unique_ids, embed_table):
    global LAST_RESULTS
    nc = _get_program()
    in_maps = _make_in_maps(neigh_cols, unique_ids, embed_table)
    trace = bool(int(os.environ.get("GCN_TRACE", "0")))
    res = run_bass_kernel_spmd(nc, in_maps, list(range(NC)), trace=trace)
    LAST_RESULTS = res
    out = np.concatenate([res.results[c]["out"] for c in range(NC)], axis=0)
    return out.astype(np.float32)


def bench_exec(inputs, iters=12):
    """Steady-state wall times (us) of the compiled NEFF via a reusable
    sharded jit with device-resident inputs. Excludes compile; includes
    per-call dispatch overhead of the runtime."""
    import time
    import jax
    from jax.sharding import Mesh, PartitionSpec, NamedSharding
    from jax.experimental.shard_map import shard_map
    from concourse.bass2jax import (_bass_exec_p, partition_id_tensor,
                                    install_neuronx_cc_hook)

    nc = _get_program()
    install_neuronx_cc_hook()
    in_maps = _make_in_maps(**inputs)

    partition_name = (nc.partition_id_tensor.name
                      if nc.partition_id_tensor else None)
    in_names, out_names, out_avals, zero_outs = [], [], [], []
    for alloc in nc.m.functions[0].allocations:
        if not isinstance(alloc, mybir.MemoryLocationSet):
            continue
        name = alloc.memorylocations[0].name
        if alloc.kind == "ExternalInput":
            if name != partition_name:
                in_names.append(name)
        elif alloc.kind == "ExternalOutput":
            out_names.append(name)
            shape = tuple(alloc.tensor_shape)
            npdt = dt.np(alloc.dtype)
            out_avals.append(jax.core.ShapedArray(shape, npdt))
            zero_outs.append(np.zeros(shape, npdt))
    n_params = len(in_names)
    all_names = in_names + out_names + ([partition_name] if partition_name else [])

    def _body(*args):
        operands = list(args)
        if partition_name is not None:
            operands.append(partition_id_tensor())
        return tuple(_bass_exec_p.bind(
            *operands, out_avals=tuple(out_avals), in_names=tuple(all_names),
            out_names=tuple(out_names), lowering_input_output_aliases=(),
            sim_require_finite=True, sim_require_nnan=True, nc=nc))

    devices = jax.devices()[:NC]
    mesh = Mesh(np.asarray(devices), ("core",))
    sharded = jax.jit(
        shard_map(_body, mesh=mesh,
                  in_specs=(PartitionSpec("core"),) * (n_params + len(out_names)),
                  out_specs=(PartitionSpec("core"),) * len(out_names),
                  check_rep=False),
        keep_unused=True)
    sh = NamedSharding(mesh, PartitionSpec("core"))
    concat_in = [jax.device_put(
        np.concatenate([np.asarray(in_maps[c][nm]) for c in range(NC)], axis=0),
        sh) for nm in in_names]
    concat_zero = [jax.device_put(
        np.zeros((NC * z.shape[0], *z.shape[1:]), z.dtype), sh)
        for z in zero_outs]
    out = sharded(*concat_in, *concat_zero)
    jax.block_until_ready(out)
    times = []
    for _ in range(iters):
        t0 = time.perf_counter()
        out = sharded(*concat_in, *concat_zero)
        jax.block_until_ready(out)
        times.append((time.perf_counter() - t0) * 1e6)
    return sorted(times)


def modeled_time_ns():
    """Single-core device-occupancy model of the program (cost-model sim)."""
    from concourse.timeline_sim import TimelineSim
    return TimelineSim(_get_program(), trace=False).simulate()
